# revision 32
# baseline (speedup 1.0000x reference)
"""Cross-attention Trainium2 kernel (8-core data-parallel over batch).

Per-core computation (one batch element per NeuronCore):
  q = x @ Wq; k = ctx @ Wk; v = ctx @ Wv
  attn = softmax((q k^T) / sqrt(dh)); out = attn @ v; y = out @ Wo + bo

v2 structure: all layout work (transposes, dtype casts, chunk-major weight
packing) happens on the host, so the device does only matmuls + softmax +
evictions:
  xT   [qd, tok]    loaded directly (host pre-transposed, bf16)
  qT   [inner, tok] = Wq_chunk^T @ xT            (bf16 in, fp32 accum)
  sT   [ctx, tok]   = kz_h^T @ qT_pair           (kz_h per-head kT zero-padded
                                                  to a 128-row stationary)
  e    [ctx, tok]   = exp(sT / 8)                (ACT; |scores/8| small enough
                                                  that max-subtraction is not
                                                  needed)
  r    [pair, tok]  = per-head column sums of e via half-ones selector
                      matmuls, written pre-broadcast across 64 partitions
  outT [dh, tok]    = v_h^T @ e, pair-packed into one PSUM bank via
                      tile_position, then * (1/r) on DVE
  y    [tok, qd]    = outT^T @ Wo + bo           (bf16 store; host upcasts)

DMA: x loads are plain bf16 loads on the gpsimd queue, weights load in
chunk-major bf16 layout (one contiguous descriptor set each, sync queue),
y stores ride the scalar queue. NOTE: putting the steady-state y stores on
the sync queue stretches every engine's instructions ~20% under 8-core HBM
concurrency (+25us) - keep them on the scalar queue.
"""

import numpy as np
import ml_dtypes

import concourse.bass as bass
import concourse.tile as tile
from concourse import bacc, mybir
from concourse.bass_utils import run_bass_kernel_spmd

F32 = mybir.dt.float32
BF16 = mybir.dt.bfloat16
NP_BF16 = ml_dtypes.bfloat16

B, N, M = 8, 4096, 77
QD, CD, H, DH = 512, 768, 8, 64
INNER = H * DH  # 512
P = 128
S = 512  # token group size
NQC = QD // P  # 4 qd chunks
NCC = CD // P  # 6 cd chunks
NIC = INNER // P  # 4 inner chunks
NTS = S // P  # 4 token sub-tiles per group
SCALE = DH ** -0.5
MP = 128  # context length padded to full partition width (zeros are inert)


def build_kernel(groups: int = N // S):
    nc = bacc.Bacc(None, target_bir_lowering=False, debug=False)

    # xT pre-packed per group: xt_d[g, p, c*S+t] = x[g*S+t, c*128+p], so each
    # group load is 128 fully-contiguous 4KB descriptors (HBM-friendly at
    # 8-core concurrency).
    xt_d = nc.dram_tensor("xT", [groups, P, NQC * S], BF16, kind="ExternalInput")
    ctxt_d = nc.dram_tensor("ctxT", [CD, MP], BF16, kind="ExternalInput")
    wq_d = nc.dram_tensor("Wq", [P, NQC * INNER], BF16, kind="ExternalInput")
    wk_d = nc.dram_tensor("Wk", [P, NCC * INNER], BF16, kind="ExternalInput")
    wv_d = nc.dram_tensor("Wv", [P, NCC * INNER], BF16, kind="ExternalInput")
    wo_d = nc.dram_tensor("Wo", [P, NIC * QD], BF16, kind="ExternalInput")
    bo_d = nc.dram_tensor("bo", [QD], F32, kind="ExternalInput")
    # y stored interleaved: y_d[g, p, ts, :] = y[g*S + ts*128 + p, :] (2KB
    # descriptor pairs); the host un-permutes.
    y_d = nc.dram_tensor("y", [groups, P, NTS, QD], BF16, kind="ExternalOutput")

    from contextlib import ExitStack

    with tile.TileContext(nc) as tc, ExitStack() as st:
        consts = st.enter_context(tc.tile_pool(name="consts", bufs=1))
        xin = st.enter_context(tc.tile_pool(name="xin", bufs=3))
        qtp = st.enter_context(tc.tile_pool(name="qt", bufs=2))
        expp = st.enter_context(tc.tile_pool(name="expp", bufs=2))
        rcp = st.enter_context(tc.tile_pool(name="rcp", bufs=3))
        outp = st.enter_context(tc.tile_pool(name="outp", bufs=3))
        yp = st.enter_context(tc.tile_pool(name="yp", bufs=3))

        # PSUM budget: 8 banks total.
        ps_q = st.enter_context(tc.tile_pool(name="ps_q", bufs=2, space="PSUM"))
        ps_s = st.enter_context(tc.tile_pool(name="ps_s", bufs=2, space="PSUM"))
        ps_rs = st.enter_context(tc.tile_pool(name="ps_rs", bufs=2, space="PSUM"))
        ps_av = st.enter_context(tc.tile_pool(name="ps_av", bufs=2, space="PSUM"))

        # ---- weight / context loads (sync queue, needed-first order) ------------
        wq_sb = consts.tile([P, NQC, INNER], BF16)
        nc.sync.dma_start(out=wq_sb, in_=wq_d.rearrange("p (c i) -> p c i", c=NQC))
        wk_sb = consts.tile([P, NCC, INNER], BF16)
        nc.scalar.dma_start(out=wk_sb, in_=wk_d.rearrange("p (c i) -> p c i", c=NCC))
        ctxT_sb = consts.tile([P, NCC, MP], BF16)
        nc.scalar.dma_start(
            out=ctxT_sb, in_=ctxt_d.rearrange("(c p) m -> p c m", p=P)
        )
        wo_sb = consts.tile([P, NIC, QD], BF16)
        nc.sync.dma_start(out=wo_sb, in_=wo_d.rearrange("p (c i) -> p c i", c=NIC))
        bo_bc = consts.tile([P, QD], F32)
        bo_ap = bo_d.ap()
        nc.sync.dma_start(
            out=bo_bc, in_=bass.AP(bo_ap.tensor, bo_ap.offset, [[0, P], [1, QD]])
        )

        # ---- x loads: plain bf16 loads, gpsimd queue, one per group -------------
        def load_x(g):
            x_g = xin.tile([P, NQC, S], BF16)
            nc.gpsimd.dma_start(
                out=x_g, in_=xt_d[g].rearrange("p (c t) -> p c t", c=NQC)
            )
            return x_g

        x_pre = [load_x(0)]
        wv_sb = consts.tile([P, NCC, INNER], BF16)
        nc.gpsimd.dma_start(out=wv_sb, in_=wv_d.rearrange("p (c i) -> p c i", c=NCC))
        x_pre.append(load_x(1))

        # rowsum selector stationaries: sel2[:, side] is [M, 128] with ones in
        # columns side*64..(side+1)*64
        sel2_stage = consts.tile([M, 2, 2, DH], F32)
        nc.vector.memset(sel2_stage, 0.0)
        nc.vector.memset(sel2_stage[:, 0, 0, :], 1.0)
        nc.vector.memset(sel2_stage[:, 1, 1, :], 1.0)
        sel2 = consts.tile([M, 2, 2, DH], BF16)
        nc.vector.tensor_copy(out=sel2, in_=sel2_stage)

        kz = consts.tile([P, H, MP], BF16)
        nc.vector.memset(kz, 0.0)

        # ---- q projection -------------------------------------------------------
        def emit_qproj(g):
            xT = x_pre[g]
            qT = qtp.tile([P, NIC, S], BF16)
            for ic in range(NIC):
                pq = ps_q.tile([P, S], F32, tag="ps_q")
                for c in range(NQC):
                    nc.tensor.matmul(
                        pq,
                        wq_sb[:, c, ic * P : (ic + 1) * P],
                        xT[:, c, :],
                        start=(c == 0),
                        stop=(c == NQC - 1),
                    )
                nc.scalar.copy(out=qT[:, ic, :], in_=pq)
            return qT

        qT_pre = [emit_qproj(0)]

        # ---- k projection: per-head kT zero-padded to full 128-row stationary ---
        for ic in range(NIC):
            pk = ps_s.tile([P, MP], F32, tag="ps_s")
            for cc in range(NCC):
                nc.tensor.matmul(
                    pk,
                    wk_sb[:, cc, ic * P : (ic + 1) * P],
                    ctxT_sb[:, cc, :],
                    start=(cc == 0),
                    stop=(cc == NCC - 1),
                )
            nc.scalar.copy(out=kz[:DH, 2 * ic, :], in_=pk[:DH, :])
            nc.scalar.copy(out=kz[DH:, 2 * ic + 1, :], in_=pk[DH:, :])

        # ---- v projection: v_sb [ctx, inner] ------------------------------------
        v_sb = consts.tile([MP, INNER], BF16)
        pv = ps_q.tile([MP, INNER], F32, tag="ps_q")
        for cc in range(NCC):
            nc.tensor.matmul(
                pv,
                ctxT_sb[:, cc, :],
                wv_sb[:, cc, :],
                start=(cc == 0),
                stop=(cc == NCC - 1),
            )
        nc.vector.tensor_copy(out=v_sb, in_=pv)

        x_pre.append(load_x(2))

        # ---- scores + exp -------------------------------------------------------
        def emit_front(g):
            qT = qT_pre[g]
            exp_g = expp.tile([MP, H, S], BF16)
            for h in range(H):
                ps_sc = ps_s.tile([MP, S], F32, tag="ps_s")
                nc.tensor.matmul(
                    ps_sc, kz[:, h, :], qT[:, h // 2, :], start=True, stop=True
                )
                nc.scalar.activation(
                    out=exp_g[:, h, :],
                    in_=ps_sc,
                    func=mybir.ActivationFunctionType.Exp,
                    scale=SCALE,
                )
            return exp_g

        exp_pre = [emit_front(0)]

        # ---- rowsums / attention-output / final projection ----------------------
        def emit_back(g):
            exp_g = exp_pre[g]
            # broadcast rowsums + reciprocal per pair
            rec_g = rcp.tile([P, H // 2, S], F32)
            for pp in range(H // 2):
                pr = ps_rs.tile([P, S], F32, tag="ps_rs")
                for side in range(2):
                    nc.tensor.matmul(
                        pr,
                        sel2[:, side],
                        exp_g[:M, 2 * pp + side, :],
                        start=(side == 0),
                        stop=(side == 1),
                    )
                nc.vector.reciprocal_approx_fast(out=rec_g[:, pp, :], in_=pr)

            # outT (unnormalized) * (1/r); pair-packed into one bank
            outT = outp.tile([P, NIC, S], BF16)
            for pp in range(H // 2):
                po = ps_av.tile([P, S], F32, tag="ps_av")
                for side in range(2):
                    h = 2 * pp + side
                    nc.tensor.matmul(
                        po[side * DH : (side + 1) * DH, :],
                        v_sb[:, h * DH : (h + 1) * DH],
                        exp_g[:, h, :],
                        start=True,
                        stop=True,
                        tile_position=(0, side * DH),
                    )
                nc.vector.tensor_mul(
                    out=outT[:, pp, :], in0=po, in1=rec_g[:, pp, :]
                )

            # final projection + bias; two half-group bf16 stores; pf
            # partition j holds token ts*128+j
            y_g = yp.tile([P, NTS, QD], BF16)
            y_ap = y_d[g]
            for ts in range(NTS):
                pf = ps_q.tile([P, QD], F32, tag="ps_q")
                for ic in range(NIC):
                    nc.tensor.matmul(
                        pf,
                        outT[:, ic, ts * P : (ts + 1) * P],
                        wo_sb[:, ic, :],
                        start=(ic == 0),
                        stop=(ic == NIC - 1),
                    )
                nc.vector.tensor_add(out=y_g[:, ts, :], in0=pf, in1=bo_bc)
                if ts % 2 == 1:
                    nc.scalar.dma_start(
                        out=y_ap[:, ts - 1 : ts + 1, :],
                        in_=y_g[:, ts - 1 : ts + 1, :],
                    )

        # ---- software-pipelined main loop ---------------------------------------
        for g in range(1, groups):
            if g + 2 < groups:
                x_pre.append(load_x(g + 2))
            qT_pre.append(emit_qproj(g))
            exp_pre.append(emit_front(g))
            emit_back(g - 1)
        emit_back(groups - 1)

    nc.compile()
    return nc


_CACHE = {}


def _get_nc():
    if "nc" not in _CACHE:
        _CACHE["nc"] = build_kernel()
    return _CACHE["nc"]


def _chunk_major(w, nchunks):
    """[nchunks*128, F] f32 -> [128, nchunks*F] bf16, chunk-major per partition."""
    f = w.shape[1]
    return np.ascontiguousarray(
        w.reshape(nchunks, P, f).transpose(1, 0, 2).reshape(P, nchunks * f)
    ).astype(NP_BF16)


def run(inputs, trace=False, **kw):
    nc = _get_nc()
    wq_h = _chunk_major(np.asarray(inputs["Wq"], np.float32), NQC)
    wk_h = _chunk_major(np.asarray(inputs["Wk"], np.float32), NCC)
    wv_h = _chunk_major(np.asarray(inputs["Wv"], np.float32), NCC)
    wo_h = _chunk_major(np.asarray(inputs["Wo"], np.float32), NIC)
    bo_h = np.asarray(inputs["bo"], np.float32)
    in_maps = []
    groups = N // S
    for i in range(B):
        # [g, p, c, t] <- x[g*S+t, c*128+p]
        xt = np.ascontiguousarray(
            np.asarray(inputs["x"][i], np.float32)
            .reshape(groups, S, NQC, P)
            .transpose(0, 3, 2, 1)
            .reshape(groups, P, NQC * S)
        ).astype(NP_BF16)
        ctx = np.zeros((CD, MP), np.float32)
        ctx[:, :M] = np.asarray(inputs["context"][i], np.float32).T
        in_maps.append(
            {
                "xT": xt,
                "ctxT": ctx.astype(NP_BF16),
                "Wq": wq_h,
                "Wk": wk_h,
                "Wv": wv_h,
                "Wo": wo_h,
                "bo": bo_h,
            }
        )
    res = run_bass_kernel_spmd(nc, in_maps, list(range(B)), trace=trace, **kw)
    # un-permute: y_d[g, p, ts, q] = y[g*S + ts*128 + p, q]
    out = np.stack(
        [
            np.asarray(res.results[i]["y"], dtype=np.float32)
            .reshape(groups, P, NTS, QD)
            .transpose(0, 2, 1, 3)
            .reshape(N, QD)
            for i in range(B)
        ],
        axis=0,
    )
    return out, res


def kernel(**inputs):
    out, _ = run(inputs)
    return out


# revision 33
# speedup vs baseline: 1.0418x; 1.0418x over previous
"""Cross-attention Trainium2 kernel (8-core data-parallel over batch).

Per-core computation (one batch element per NeuronCore):
  q = x @ Wq; k = ctx @ Wk; v = ctx @ Wv
  attn = softmax((q k^T) / sqrt(dh)); out = attn @ v; y = out @ Wo + bo

v2 structure: all layout work (transposes, dtype casts, chunk-major weight
packing) happens on the host, so the device does only matmuls + softmax +
evictions:
  xT   [qd, tok]    loaded directly (host pre-transposed, bf16)
  qT   [inner, tok] = Wq_chunk^T @ xT            (bf16 in, fp32 accum)
  sT   [ctx, tok]   = kz_h^T @ qT_pair           (kz_h per-head kT zero-padded
                                                  to a 128-row stationary)
  e    [ctx, tok]   = exp(sT / 8)                (ACT; |scores/8| small enough
                                                  that max-subtraction is not
                                                  needed)
  r    [pair, tok]  = per-head column sums of e via half-ones selector
                      matmuls, written pre-broadcast across 64 partitions
  outT [dh, tok]    = v_h^T @ e, pair-packed into one PSUM bank via
                      tile_position, then * (1/r) on DVE
  y    [tok, qd]    = outT^T @ Wo + bo           (bf16 store; host upcasts)

DMA: x loads are plain bf16 loads on the gpsimd queue, weights load in
chunk-major bf16 layout (one contiguous descriptor set each, sync queue),
y stores ride the scalar queue. NOTE: putting the steady-state y stores on
the sync queue stretches every engine's instructions ~20% under 8-core HBM
concurrency (+25us) - keep them on the scalar queue.
"""

import numpy as np
import ml_dtypes

import concourse.bass as bass
import concourse.tile as tile
from concourse import bacc, mybir
from concourse.bass_utils import run_bass_kernel_spmd

F32 = mybir.dt.float32
BF16 = mybir.dt.bfloat16
NP_BF16 = ml_dtypes.bfloat16

B, N, M = 8, 4096, 77
QD, CD, H, DH = 512, 768, 8, 64
INNER = H * DH  # 512
P = 128
S = 512  # token group size
NQC = QD // P  # 4 qd chunks
NCC = CD // P  # 6 cd chunks
NIC = INNER // P  # 4 inner chunks
NTS = S // P  # 4 token sub-tiles per group
SCALE = DH ** -0.5
MP = 128  # context length padded to full partition width (zeros are inert)


def build_kernel(groups: int = N // S):
    nc = bacc.Bacc(None, target_bir_lowering=False, debug=False)

    # xT pre-packed per group: xt_d[g, p, c*S+t] = x[g*S+t, c*128+p], so each
    # group load is 128 fully-contiguous 4KB descriptors (HBM-friendly at
    # 8-core concurrency).
    xt_d = nc.dram_tensor("xT", [groups, P, NQC * S], BF16, kind="ExternalInput")
    ctxt_d = nc.dram_tensor("ctxT", [CD, MP], BF16, kind="ExternalInput")
    wq_d = nc.dram_tensor("Wq", [P, NQC * INNER], BF16, kind="ExternalInput")
    wk_d = nc.dram_tensor("Wk", [P, NCC * INNER], BF16, kind="ExternalInput")
    wv_d = nc.dram_tensor("Wv", [P, NCC * INNER], BF16, kind="ExternalInput")
    wo_d = nc.dram_tensor("Wo", [P, NIC * QD], BF16, kind="ExternalInput")
    bo_d = nc.dram_tensor("bo", [QD], F32, kind="ExternalInput")
    # y stored interleaved: y_d[g, p, ts, :] = y[g*S + ts*128 + p, :] (2KB
    # descriptor pairs); the host un-permutes.
    y_d = nc.dram_tensor("y", [groups, P, NTS, QD], BF16, kind="ExternalOutput")

    from contextlib import ExitStack

    with tile.TileContext(nc) as tc, ExitStack() as st:
        consts = st.enter_context(tc.tile_pool(name="consts", bufs=1))
        xin = st.enter_context(tc.tile_pool(name="xin", bufs=3))
        qtp = st.enter_context(tc.tile_pool(name="qt", bufs=2))
        expp = st.enter_context(tc.tile_pool(name="expp", bufs=2))
        rcp = st.enter_context(tc.tile_pool(name="rcp", bufs=3))
        outp = st.enter_context(tc.tile_pool(name="outp", bufs=3))
        yp = st.enter_context(tc.tile_pool(name="yp", bufs=3))

        # PSUM budget: 8 banks total.
        ps_q = st.enter_context(tc.tile_pool(name="ps_q", bufs=2, space="PSUM"))
        ps_s = st.enter_context(tc.tile_pool(name="ps_s", bufs=2, space="PSUM"))
        ps_rs = st.enter_context(tc.tile_pool(name="ps_rs", bufs=2, space="PSUM"))
        ps_av = st.enter_context(tc.tile_pool(name="ps_av", bufs=2, space="PSUM"))

        # ---- weight / context loads (sync queue, needed-first order) ------------
        wq_sb = consts.tile([P, NQC, INNER], BF16)
        nc.sync.dma_start(out=wq_sb, in_=wq_d.rearrange("p (c i) -> p c i", c=NQC))
        wk_sb = consts.tile([P, NCC, INNER], BF16)
        nc.sync.dma_start(out=wk_sb, in_=wk_d.rearrange("p (c i) -> p c i", c=NCC))
        ctxT_sb = consts.tile([P, NCC, MP], BF16)
        nc.sync.dma_start(
            out=ctxT_sb, in_=ctxt_d.rearrange("(c p) m -> p c m", p=P)
        )
        wv_sb = consts.tile([P, NCC, INNER], BF16)
        nc.sync.dma_start(out=wv_sb, in_=wv_d.rearrange("p (c i) -> p c i", c=NCC))
        wo_sb = consts.tile([P, NIC, QD], BF16)
        nc.sync.dma_start(out=wo_sb, in_=wo_d.rearrange("p (c i) -> p c i", c=NIC))
        bo_bc = consts.tile([P, QD], F32)
        bo_ap = bo_d.ap()
        nc.sync.dma_start(
            out=bo_bc, in_=bass.AP(bo_ap.tensor, bo_ap.offset, [[0, P], [1, QD]])
        )

        # ---- x loads: plain bf16 loads, gpsimd queue, one per group -------------
        def load_x(g):
            x_g = xin.tile([P, NQC, S], BF16)
            nc.gpsimd.dma_start(
                out=x_g, in_=xt_d[g].rearrange("p (c t) -> p c t", c=NQC)
            )
            return x_g

        x_pre = [load_x(0), load_x(1)]

        # rowsum selector stationaries: sel2[:, side] is [M, 128] with ones in
        # columns side*64..(side+1)*64
        sel2_stage = consts.tile([M, 2, 2, DH], F32)
        nc.vector.memset(sel2_stage, 0.0)
        nc.vector.memset(sel2_stage[:, 0, 0, :], 1.0)
        nc.vector.memset(sel2_stage[:, 1, 1, :], 1.0)
        sel2 = consts.tile([M, 2, 2, DH], BF16)
        nc.vector.tensor_copy(out=sel2, in_=sel2_stage)

        kz = consts.tile([P, H, MP], BF16)
        nc.vector.memset(kz, 0.0)

        # ---- q projection -------------------------------------------------------
        def emit_qproj(g):
            xT = x_pre[g]
            qT = qtp.tile([P, NIC, S], BF16)
            for ic in range(NIC):
                pq = ps_q.tile([P, S], F32, tag="ps_q")
                for c in range(NQC):
                    nc.tensor.matmul(
                        pq,
                        wq_sb[:, c, ic * P : (ic + 1) * P],
                        xT[:, c, :],
                        start=(c == 0),
                        stop=(c == NQC - 1),
                    )
                nc.scalar.copy(out=qT[:, ic, :], in_=pq)
            return qT

        qT_pre = [emit_qproj(0)]

        # ---- k projection: per-head kT zero-padded to full 128-row stationary ---
        for ic in range(NIC):
            pk = ps_s.tile([P, MP], F32, tag="ps_s")
            for cc in range(NCC):
                nc.tensor.matmul(
                    pk,
                    wk_sb[:, cc, ic * P : (ic + 1) * P],
                    ctxT_sb[:, cc, :],
                    start=(cc == 0),
                    stop=(cc == NCC - 1),
                )
            nc.scalar.copy(out=kz[:DH, 2 * ic, :], in_=pk[:DH, :])
            nc.scalar.copy(out=kz[DH:, 2 * ic + 1, :], in_=pk[DH:, :])

        # ---- v projection: v_sb [ctx, inner] ------------------------------------
        v_sb = consts.tile([MP, INNER], BF16)
        pv = ps_q.tile([MP, INNER], F32, tag="ps_q")
        for cc in range(NCC):
            nc.tensor.matmul(
                pv,
                ctxT_sb[:, cc, :],
                wv_sb[:, cc, :],
                start=(cc == 0),
                stop=(cc == NCC - 1),
            )
        nc.vector.tensor_copy(out=v_sb, in_=pv)

        x_pre.append(load_x(2))

        # ---- scores + exp -------------------------------------------------------
        def emit_front(g):
            qT = qT_pre[g]
            exp_g = expp.tile([MP, H, S], BF16)
            for h in range(H):
                ps_sc = ps_s.tile([MP, S], F32, tag="ps_s")
                nc.tensor.matmul(
                    ps_sc, kz[:, h, :], qT[:, h // 2, :], start=True, stop=True
                )
                nc.scalar.activation(
                    out=exp_g[:, h, :],
                    in_=ps_sc,
                    func=mybir.ActivationFunctionType.Exp,
                    scale=SCALE,
                )
            return exp_g

        exp_pre = [emit_front(0)]

        # ---- rowsums / attention-output / final projection ----------------------
        def emit_back(g):
            exp_g = exp_pre[g]
            # broadcast rowsums + reciprocal per pair
            rec_g = rcp.tile([P, H // 2, S], F32)
            for pp in range(H // 2):
                pr = ps_rs.tile([P, S], F32, tag="ps_rs")
                for side in range(2):
                    nc.tensor.matmul(
                        pr,
                        sel2[:, side],
                        exp_g[:M, 2 * pp + side, :],
                        start=(side == 0),
                        stop=(side == 1),
                    )
                nc.vector.reciprocal_approx_fast(out=rec_g[:, pp, :], in_=pr)

            # outT (unnormalized) * (1/r); pair-packed into one bank
            outT = outp.tile([P, NIC, S], BF16)
            for pp in range(H // 2):
                po = ps_av.tile([P, S], F32, tag="ps_av")
                for side in range(2):
                    h = 2 * pp + side
                    nc.tensor.matmul(
                        po[side * DH : (side + 1) * DH, :],
                        v_sb[:, h * DH : (h + 1) * DH],
                        exp_g[:, h, :],
                        start=True,
                        stop=True,
                        tile_position=(0, side * DH),
                    )
                nc.vector.tensor_mul(
                    out=outT[:, pp, :], in0=po, in1=rec_g[:, pp, :]
                )

            # final projection + bias; two half-group bf16 stores; pf
            # partition j holds token ts*128+j
            y_g = yp.tile([P, NTS, QD], BF16)
            y_ap = y_d[g]
            for ts in range(NTS):
                pf = ps_q.tile([P, QD], F32, tag="ps_q")
                for ic in range(NIC):
                    nc.tensor.matmul(
                        pf,
                        outT[:, ic, ts * P : (ts + 1) * P],
                        wo_sb[:, ic, :],
                        start=(ic == 0),
                        stop=(ic == NIC - 1),
                    )
                nc.vector.tensor_add(out=y_g[:, ts, :], in0=pf, in1=bo_bc)
                if ts % 2 == 1:
                    nc.scalar.dma_start(
                        out=y_ap[:, ts - 1 : ts + 1, :],
                        in_=y_g[:, ts - 1 : ts + 1, :],
                    )

        # ---- software-pipelined main loop ---------------------------------------
        for g in range(1, groups):
            if g + 2 < groups:
                x_pre.append(load_x(g + 2))
            qT_pre.append(emit_qproj(g))
            exp_pre.append(emit_front(g))
            emit_back(g - 1)
        emit_back(groups - 1)

    nc.compile()
    return nc


_CACHE = {}


def _get_nc():
    if "nc" not in _CACHE:
        _CACHE["nc"] = build_kernel()
    return _CACHE["nc"]


def _chunk_major(w, nchunks):
    """[nchunks*128, F] f32 -> [128, nchunks*F] bf16, chunk-major per partition."""
    f = w.shape[1]
    return np.ascontiguousarray(
        w.reshape(nchunks, P, f).transpose(1, 0, 2).reshape(P, nchunks * f)
    ).astype(NP_BF16)


def run(inputs, trace=False, **kw):
    nc = _get_nc()
    wq_h = _chunk_major(np.asarray(inputs["Wq"], np.float32), NQC)
    wk_h = _chunk_major(np.asarray(inputs["Wk"], np.float32), NCC)
    wv_h = _chunk_major(np.asarray(inputs["Wv"], np.float32), NCC)
    wo_h = _chunk_major(np.asarray(inputs["Wo"], np.float32), NIC)
    bo_h = np.asarray(inputs["bo"], np.float32)
    in_maps = []
    groups = N // S
    for i in range(B):
        # [g, p, c, t] <- x[g*S+t, c*128+p]
        xt = np.ascontiguousarray(
            np.asarray(inputs["x"][i], np.float32)
            .reshape(groups, S, NQC, P)
            .transpose(0, 3, 2, 1)
            .reshape(groups, P, NQC * S)
        ).astype(NP_BF16)
        ctx = np.zeros((CD, MP), np.float32)
        ctx[:, :M] = np.asarray(inputs["context"][i], np.float32).T
        in_maps.append(
            {
                "xT": xt,
                "ctxT": ctx.astype(NP_BF16),
                "Wq": wq_h,
                "Wk": wk_h,
                "Wv": wv_h,
                "Wo": wo_h,
                "bo": bo_h,
            }
        )
    res = run_bass_kernel_spmd(nc, in_maps, list(range(B)), trace=trace, **kw)
    # un-permute: y_d[g, p, ts, q] = y[g*S + ts*128 + p, q]
    out = np.stack(
        [
            np.asarray(res.results[i]["y"], dtype=np.float32)
            .reshape(groups, P, NTS, QD)
            .transpose(0, 2, 1, 3)
            .reshape(N, QD)
            for i in range(B)
        ],
        axis=0,
    )
    return out, res


def kernel(**inputs):
    out, _ = run(inputs)
    return out
